# revision 2
# baseline (speedup 1.0000x reference)
"""Trainium2 Bass kernel for nn_DevConv (gnn_message_passing, N=8192).

Math (reference): per node i,
  maxd2[i] = relu(max over {j: adj[i,j]>0} of ||w*(x_i-x_j)||^2)
  out[i]   = 0.5*(prev[i] + mean(W_phi)*sqrt(maxd2[i]))

v2 design (vs v1 which shipped int32 adjacency + ACT cast, 126.7us):
  * adjacency shipped as 1-byte fp8 e4m3 ({0,1} exact) -> 8 MiB/core of HBM
    traffic instead of 32 MiB (DMA roofline ~358 GB/s/NC -> ~23.4us).
  * node dim sharded across 8 cores; per core 9 i-tiles (8x117 + 88 rows),
    each tile's [mt, 8192] fp8 slab DMA'd straight into the matmul rhs
    buffer (partitions 0..116; partitions 117..127 hold constant fp8 y-rows).
    No ACT cast pass at all, and zero DMA overlap waste.
  * ONE fp8 matmul per 512-col chunk, K=128:
      lhsT = [BIG*I(mt) ; 0 ; y-lhs rows]  (fp8 e4m3)
      psum[i,j] = BIG*adj[i,j] + sq_j - 2*y_i.y_j      (y normalized: d2<=1)
    y encoded as 2-way e4m3 split (9 product rows + 2 sq rows = 11 rows,
    ~1e-3 abs err on d2).
  * PSUM drained by TWO engines in parallel (the v1 bottleneck was DVE-only
    drain; any PSUM read is 1 elem/cycle/lane):
      - DVE: tensor_reduce(max) on even 1024-col waves -> exact max
      - ACT: Exp(scale*psum + bias) with accum_out on odd waves -> per-row
        sum of exp(BETA*(v - C_i)), i.e. a log-sum-exp max with overshoot
        <= log(k_ties)/BETA (~0.03 abs on a <=1-scale d2).
    4 psum regions of [128,1024] (2 DVE + 2 ACT, ping-pong) = all 8 banks.
  * Host epilogue O(N): fold group maxes / LSE sums, relu, sqrt, rescale.

The BIG-offset mask is exact: BIG=2 > max d2'=1, rows with a neighbor give
BIG + max_nb d2'; rows without stay < BIG and the final relu clamps to 0.
"""
from contextlib import ExitStack

import numpy as np
import ml_dtypes

import concourse.bacc as bacc
from concourse import mybir
from concourse.bass_utils import run_bass_kernel_spmd

F8 = ml_dtypes.float8_e4m3  # TRN FP8_EXP4 (matches for |v| <= 240)

N = 8192
CORES = 8
ROWS = N // CORES            # 1024 rows per core
MT = 117                     # i-rows per full tile (128 - 11 y rows)
TILES = 9                    # 8 x 117 + 88 = 1024
K_Y = 11                     # y contraction rows (2-way e4m3 split)
Y_P0 = 117                   # partitions holding y rows (117..127)
CHUNK = 512                  # matmul free dim (1 psum bank fp32)
WAVE = 1024                  # cols per drain wave (2 chunks, 2 banks)
WPT = N // WAVE              # 8 waves per tile: D,A,D,A,D,A,D,A
DPT = WPT // 2               # 4 DVE waves / tile
APT = WPT // 2               # 4 ACT waves / tile
BIG = 2.0
BETA = 84.0                  # LSE sharpness; exp(-BETA) stays normal in fp32

_NC = {}


def _tile_rows(t):
    return MT if t < TILES - 1 else ROWS - MT * (TILES - 1)


def _build_nc(reps=1, stage="full"):
    """Per-core program. reps>1 replays the pipeline on the same inputs (for
    HW-time measurement via wall-clock deltas). stage in {dma, pe, full}."""
    if (reps, stage) in _NC:
        return _NC[(reps, stage)]
    nc = bacc.Bacc("TRN2", target_bir_lowering=False, debug=False, num_devices=CORES)

    adj_d = nc.declare_dram_parameter("adj", [ROWS, N], mybir.dt.uint8, isOutput=False)
    lhsT_d = nc.declare_dram_parameter(
        "lhsT", [128, TILES * MT], mybir.dt.uint8, isOutput=False
    )
    yT_d = nc.declare_dram_parameter("yT", [K_Y, N], mybir.dt.uint8, isOutput=False)
    bias_d = nc.declare_dram_parameter(
        "bias", [128, TILES], mybir.dt.float32, isOutput=False
    )
    gmax_d = nc.declare_dram_parameter(
        "gmax", [128, 2 * TILES * DPT], mybir.dt.float32, isOutput=True
    )

    NBUF = 3
    bufs = [nc.alloc_sbuf_tensor(f"buf{i}", [128, N], mybir.dt.uint8) for i in range(NBUF)]
    lhsT_sb = nc.alloc_sbuf_tensor("lhsTsb", [128, TILES * MT], mybir.dt.uint8)
    bias_sb = nc.alloc_sbuf_tensor("biassb", [128, TILES], mybir.dt.float32)
    acc_sb = nc.alloc_sbuf_tensor("accsb", [128, 2 * TILES * DPT], mybir.dt.float32)
    ps_d = [nc.alloc_psum_tensor(f"psd{i}", [128, WAVE], mybir.dt.float32) for i in range(2)]
    ps_a = [nc.alloc_psum_tensor(f"psa{i}", [128, WAVE], mybir.dt.float32) for i in range(2)]

    NGD = TILES * DPT          # 36 DVE waves per rep
    NGA = TILES * APT          # 36 ACT waves per rep
    ACOL = NGD                 # ACT accum cols start at 36

    f8 = mybir.dt.float8e4

    with ExitStack() as es:
        block = es.enter_context(nc.Block())
        const_sem = es.enter_context(nc.semaphore("const_sem"))
        a_sems = [es.enter_context(nc.semaphore(f"a_sem{t}")) for t in range(TILES)]
        pe_d_sem = es.enter_context(nc.semaphore("pe_d_sem"))
        pe_a_sem = es.enter_context(nc.semaphore("pe_a_sem"))
        dve_sem = es.enter_context(nc.semaphore("dve_sem"))
        act_sem = es.enter_context(nc.semaphore("act_sem"))
        out_sem = es.enter_context(nc.semaphore("out_sem"))

        NT = TILES * reps
        has_pe = stage in ("pe", "full")
        has_drain = stage == "full"

        @block.sync
        def _(sp):
            sp.dma_start(out=lhsT_sb[:, :], in_=lhsT_d[:, :]).then_inc(const_sem, 16)
            sp.dma_start(out=bias_sb[:, :], in_=bias_d[:, :]).then_inc(const_sem, 16)
            for b in range(NBUF):
                sp.dma_start(
                    out=bufs[b][Y_P0 : Y_P0 + K_Y, :], in_=yT_d[:, :]
                ).then_inc(const_sem, 16)
            for T in range(NT):
                t = T % TILES
                mt = _tile_rows(t)
                if T >= NBUF and has_pe:
                    # buffer slot T%NBUF free once PE finished tile T-NBUF
                    sp.wait_ge(pe_a_sem, APT * (T - NBUF) + APT)
                sp.dma_start(
                    out=bufs[T % NBUF][0:mt, :], in_=adj_d[t * MT : t * MT + mt, :]
                ).then_inc(a_sems[t], 16)
            if has_drain:
                sp.wait_ge(dve_sem, NGD * reps)
                sp.wait_ge(act_sem, NGA * reps)
            elif has_pe:
                sp.wait_ge(pe_d_sem, NGD * reps)
                sp.wait_ge(pe_a_sem, NGA * reps)
            else:
                for t in range(TILES):
                    sp.wait_ge(a_sems[t], 16 * reps)
            sp.dma_start(out=gmax_d[:, :], in_=acc_sb[:, :]).then_inc(out_sem, 16)
            sp.wait_ge(out_sem, 16)

        if has_pe:

            @block.tensor
            def _(pe):
                pe.wait_ge(const_sem, 16 * (2 + NBUF))
                for T in range(NT):
                    t = T % TILES
                    mt = _tile_rows(t)
                    lhsT = lhsT_sb[:, t * MT : t * MT + mt].bitcast(f8)
                    pe.wait_ge(a_sems[t], 16 * (T // TILES + 1))
                    for w in range(WPT):
                        if w % 2 == 0:
                            k = T * DPT + w // 2
                            ps, sem, fsem = ps_d[k % 2], pe_d_sem, dve_sem
                        else:
                            k = T * APT + w // 2
                            ps, sem, fsem = ps_a[k % 2], pe_a_sem, act_sem
                        if k >= 2 and has_drain:
                            # psum region k%2 free once its drain engine
                            # consumed wave k-2
                            pe.wait_ge(fsem, k - 1)
                        base = w * WAVE
                        mm = None
                        for c in range(WAVE // CHUNK):
                            mm = pe.matmul(
                                ps[0:mt, c * CHUNK : (c + 1) * CHUNK],
                                lhsT,
                                bufs[T % NBUF][:, base + c * CHUNK : base + (c + 1) * CHUNK].bitcast(f8),
                                start=True,
                                stop=True,
                            )
                        mm.then_inc(sem)

        if has_drain:

            @block.vector
            def _(dve):
                for k in range(NGD * reps):
                    t = (k // DPT) % TILES
                    mt = _tile_rows(t)
                    dve.wait_ge(pe_d_sem, k + 1)
                    dve.tensor_reduce(
                        out=acc_sb[0:mt, k % NGD : k % NGD + 1],
                        in_=ps_d[k % 2][0:mt, :],
                        axis=mybir.AxisListType.X,
                        op=mybir.AluOpType.max,
                    ).then_inc(dve_sem)

            @block.scalar
            def _(act):
                act.memzero(acc_sb[:, ACOL : 2 * ACOL])
                for k in range(NGA * reps):
                    t = (k // APT) % TILES
                    mt = _tile_rows(t)
                    act.wait_ge(pe_a_sem, k + 1)
                    act.activation(
                        out=ps_a[k % 2][0:mt, :],
                        in_=ps_a[k % 2][0:mt, :],
                        func=mybir.ActivationFunctionType.Exp,
                        bias=bias_sb[0:mt, t : t + 1],
                        scale=BETA,
                        accum_out=acc_sb[0:mt, ACOL + k % NGA : ACOL + k % NGA + 1],
                    ).then_inc(act_sem)

    nc.compile()
    _NC[(reps, stage)] = nc
    return nc


def _split2(v):
    """2-way e4m3 split: v ~= h + l (~1e-3 abs residual for |v|<=1)."""
    h = v.astype(F8)
    l = (v - h.astype(np.float32)).astype(F8)
    return h, l


def _build_rows(y, sq):
    """y-side lhs rows [11, n] (columns = node i, already * -2) and rhs rows
    [11, n] (columns = j): sum_k lhs[k,i]*rhs[k,j] ~= sq_j - 2 y_i.y_j."""
    n = y.shape[0]
    bh, bl = _split2(y)
    b = {"h": bh, "l": bl}
    sh, sl = _split2(sq)
    ones = np.ones(n, dtype=F8)

    pairs = [("h", "h"), ("h", "l"), ("l", "h")]
    lhs_rows, rhs_rows = [], []
    for c in range(3):
        for p1, p2 in pairs:
            lhs_rows.append((-2.0 * b[p1][:, c].astype(np.float32)).astype(F8))
            rhs_rows.append(b[p2][:, c])
    for s_part in (sh, sl):
        lhs_rows.append(ones)
        rhs_rows.append(s_part)
    return np.stack(lhs_rows, axis=0), np.stack(rhs_rows, axis=0)


def _prepare(previous_inclusion_score, nodes, adjacency_matrix, W_phi, W_theta):
    prev = np.asarray(previous_inclusion_score, dtype=np.float32)
    nodes = np.asarray(nodes, dtype=np.float32)
    adj = np.asarray(adjacency_matrix)
    W_phi = np.asarray(W_phi, dtype=np.float32)
    w = np.asarray(W_theta, dtype=np.float32)[:, 0]

    y0 = (nodes * w[None, :]).astype(np.float32)
    # normalize so max possible d2 = (2*max|y|)^2 = 1  ->  d2' <= 1, sq' <= 1/4
    s_norm = np.float32(1.0 / (2.0 * np.sqrt((y0 * y0).sum(axis=1)).max()))
    y = y0 * s_norm
    sq = np.sum(y * y, axis=1, dtype=np.float32)

    # adjacency {0,1} as fp8 e4m3 bytes (1.0 = 0x38)
    adj_f8 = ((adj != 0).astype(np.uint8) * np.uint8(0x38))

    ylhs, yT = _build_rows(y, sq)                       # [11, N] e4m3
    yT_u8 = np.ascontiguousarray(yT.view(np.uint8))
    eye = (np.eye(MT, dtype=np.float32) * np.float32(BIG)).astype(F8)

    # bias_i = -BETA * C_i with C_i = BIG + 1 - sq_i'  (per-partition, per tile)
    bias_all = (-BETA * (BIG + 1.0 - sq)).astype(np.float32)

    in_maps = []
    for k in range(CORES):
        lhsT_all = np.zeros((128, TILES * MT), dtype=F8)
        bias_k = np.zeros((128, TILES), dtype=np.float32)
        for t in range(TILES):
            mt = _tile_rows(t)
            cols = slice(t * MT, t * MT + mt)
            lhsT_all[0:mt, cols] = eye[0:mt, 0:mt]
            node_lo = k * ROWS + t * MT
            lhsT_all[Y_P0:128, cols] = ylhs[:, node_lo : node_lo + mt]
            bias_k[0:mt, t] = bias_all[node_lo : node_lo + mt]
        in_maps.append(
            {
                "adj": adj_f8[k * ROWS : (k + 1) * ROWS],
                "lhsT": np.ascontiguousarray(lhsT_all.view(np.uint8)),
                "yT": yT_u8,
                "bias": bias_k,
            }
        )
    return in_maps, prev, sq, s_norm, W_phi


def _finish(res, prev, sq, s_norm, W_phi):
    NGD = TILES * DPT
    m = np.empty(N, dtype=np.float32)
    for k in range(CORES):
        gm = res.results[k]["gmax"].astype(np.float64)   # [128, 72]
        for t in range(TILES):
            mt = _tile_rows(t)
            lo = k * ROWS + t * MT
            dmax = gm[0:mt, t * DPT : (t + 1) * DPT].max(axis=1)
            ssum = gm[0:mt, NGD + t * APT : NGD + (t + 1) * APT].sum(axis=1)
            sq_t = sq[lo : lo + mt].astype(np.float64)
            c_i = BIG + 1.0 - sq_t
            with np.errstate(divide="ignore"):
                lse = np.where(ssum > 0, c_i + np.log(ssum) / BETA, -np.inf)
            m[lo : lo + mt] = np.maximum(dmax, lse)

    maxd2 = np.maximum(m + sq - np.float32(BIG), 0.0)
    max_dist = np.sqrt(maxd2) / s_norm
    inc_mean = (max_dist * W_phi.mean()).astype(np.float32)
    return ((prev + inc_mean) * 0.5).astype(np.float32)


def kernel(previous_inclusion_score, nodes, adjacency_matrix, W_phi, W_theta):
    in_maps, prev, sq, s_norm, W_phi = _prepare(
        previous_inclusion_score, nodes, adjacency_matrix, W_phi, W_theta
    )
    nc = _build_nc()
    res = run_bass_kernel_spmd(nc, in_maps, list(range(CORES)))
    return _finish(res, prev, sq, s_norm, W_phi)
